# revision 17
# baseline (speedup 1.0000x reference)
"""MoE block (top-2 routed 3x3 conv experts) Trainium2 Bass kernel — v4.

Data-parallel over batch, 2 samples per core on 8 cores. The conv is
linear in the kernel, so the top-2 expert kernels are combined with the
routing probabilities first (w_comb = sum_e p_e W_e + I, the +I folding
the residual into the center tap), then one 3x3 SAME conv per sample.

Conv-as-matmul, 3 matmuls per 3-row tile: x lives zero-padded
[130x130] per channel in SBUF (bf16), partitions 0-63 = channels,
partitions 64-127 = same channels shifted +2 pixels. lhsT is
[128K, 128M]: M 0:64 ("A") = taps dx=0 (top K) / dx=2 (bottom K);
M 64:128 ("B") = center dx=1 taps (top K, bottom zero). The 3 dy reads
(390 cols each, dyi-major for LDWEIGHTS reuse) accumulate into one
PSUM bank per tile. Post-op: ACT stages psB (+1 col) to SBUF bf16 (DVE
may read only ONE PSUM operand, NCC_IBVF027), then one DVE
scalar_tensor_tensor per 2-tile pair: out = psA + b_comb + sbB.

DMA facts that shape the layout: queues are descriptor-rate-bound
(~4.3ns/desc) AND a transfer spanning only 64 partitions runs at HALF
rate. So the host ships x as [128, 16900] per sample — both SBUF
halves (padded flat + 2-px-shifted copy) prebuilt — making every x
chunk one full-rate 128-partition contiguous DMA (64B-descriptor-free).
The output is likewise written as [128, 9360]: 24-row batches packed
into alternating partition halves (host strips pads / reassembles).

Lanes: SP = s0 x chunks 0-1, wps weights, h_ext consts, out DMAs;
gpsimd = gconst, s0 x chunks 2-3, all s1 x chunks; ACT = s0-top GAP,
exp, B-half stagings; DVE = s0-bot GAP, gates, MAC chains, s1 GAP
(spread between pair combines), pair combines.
"""
import numpy as np
from contextlib import ExitStack

import ml_dtypes

import concourse.bass as bass
import concourse.tile as tile
from concourse import bacc, mybir
from concourse.bass_utils import run_bass_kernel_spmd

F32 = mybir.dt.float32
BF16 = mybir.dt.bfloat16
AX = mybir.AxisListType
OP = mybir.AluOpType
ACTF = mybir.ActivationFunctionType

B, C, H, W, E, GH = 16, 64, 128, 128, 8, 16
NCORES = 8
SPB = B // NCORES          # samples per core
HP, WP = H + 2, W + 2      # 130
FLAT = HP * WP             # 16900
QC = FLAT // 4             # x-load chunk size (4225 flat elements)
OBW = 24 * WP              # out batch region width (3120)
TILES = [(3 * t, 3) for t in range(42)] + [(126, 2)]
PAIRS = [(2 * p, 2 * p + 1) for p in range(21)] + [(42,)]
GATE_SPLIT = 13            # emit s1 gate work after this many s0 pairs

NPBF16 = ml_dtypes.bfloat16

_cache = {}

# GAP windows over the flat layout (pad zeros included): top covers x
# rows 0..63 (+ the first elem of x row 64 that the shifted bottom copy
# misses), bottom covers x rows 64..127.
GAP_TOP = [(0, QC, 0), (QC, 2 * QC + 2, 1)]
GAP_BOT = [(2 * QC, 3 * QC, 0), (3 * QC, FLAT, 1)]


def _emit_gap_op(nc, pools, XX, part, win, is_bot, eng):
    a, b, slot = win
    lo, hi = (64, 128) if is_bot else (0, 64)
    src = XX[lo:hi, a:b]
    dst = pools["scrS" if eng == "act" else "scrD"][lo:hi]
    acc = part[lo:hi, slot : slot + 1]
    if eng == "act":
        nc.scalar.activation(dst[:, 0 : b - a], src, ACTF.Copy, accum_out=acc)
    else:
        nc.vector.tensor_scalar(
            dst[:, 0 : b - a], src, 0.0, 0.0, OP.add, OP.add, accum_out=acc
        )


def _emit_gate(nc, pools, s, pooled, consts, h_ext):
    """Gate MLP + softmax + top-2 for one sample (all f32).

    exp-without-max-sub (logits are small); folds the top-2 mask and
    renormalization: w8 = (u>=m2)*u / (sum((u>=m2)*u) + sum(u)*1e-8).
    Returns (wb_sb [128,E] f32 per-partition probs, b_comb [C,1]).
    """
    f = pools
    g = f["gate"]
    wg1x2_sb, bg1_sb, wg2_sb, bexp_sb, ones = consts
    n = lambda base: f"{base}{s}"

    h_ps = f["gpsum"].tile([GH, 1], F32, tag="gps", name=n("h_ps"))
    nc.tensor.matmul(h_ps[:], lhsT=wg1x2_sb, rhs=pooled[:], start=True, stop=True)
    nc.vector.tensor_scalar(h_ext[0:GH, :], h_ps[:], bg1_sb, 0.0, OP.add, OP.max)

    lg_ps = f["gpsum"].tile([1, E], F32, tag="gps", name=n("lg_ps"))
    nc.tensor.matmul(lg_ps[:], lhsT=h_ext[:], rhs=wg2_sb, start=True, stop=True)

    u = g.tile([1, E], F32, tag="u", name=n("u"))
    nc.scalar.activation(u[:], lg_ps[:], ACTF.Exp)
    usum = g.tile([1, 1], F32, tag="usum", name=n("usum"))
    nc.vector.tensor_reduce(usum[:], u[:], axis=AX.X, op=OP.add)
    m1p = g.tile([1, 1], F32, tag="m1p", name=n("m1p"))
    nc.vector.tensor_reduce(m1p[:], u[:], axis=AX.X, op=OP.max)
    pm = g.tile([1, E], F32, tag="pm", name=n("pm"))
    nc.vector.scalar_tensor_tensor(pm[:], u[:], m1p[:], u[:], op0=OP.is_lt, op1=OP.mult)
    m2 = g.tile([1, 1], F32, tag="m2", name=n("m2"))
    nc.vector.tensor_reduce(m2[:], pm[:], axis=AX.X, op=OP.max)
    spv = g.tile([1, E], F32, tag="spv", name=n("spv"))
    nc.vector.scalar_tensor_tensor(spv[:], u[:], m2[:], u[:], op0=OP.is_ge, op1=OP.mult)
    dsum = g.tile([1, 1], F32, tag="dsum", name=n("dsum"))
    nc.vector.tensor_reduce(dsum[:], spv[:], axis=AX.X, op=OP.add)
    dd = g.tile([1, 1], F32, tag="dd", name=n("dd"))
    nc.vector.scalar_tensor_tensor(dd[:], usum[:], 1e-8, dsum[:], op0=OP.mult, op1=OP.add)
    rr = g.tile([1, 1], F32, tag="rr", name=n("rr"))
    nc.vector.reciprocal(rr[:], dd[:])
    w8 = g.tile([1, E], F32, tag="w8", name=n("w8"))
    nc.vector.tensor_scalar_mul(w8[:], spv[:], rr[:])

    # broadcast w8 down all 128 partitions, then stage to SBUF for MACs
    wb_ps = f["gpsum"].tile([128, E], F32, tag="gps", name=n("wb_ps"))
    nc.tensor.matmul(wb_ps[:], lhsT=ones[:], rhs=w8[:], start=True, stop=True)
    wb_sb = g.tile([128, E], F32, tag="wb_sb", name=n("wb_sb"))
    nc.vector.tensor_copy(wb_sb[:], wb_ps[:])

    # combined bias: b_comb = b_exp^T @ w8^T
    w8c_ps = f["gpsum"].tile([E, 1], F32, tag="gps", name=n("w8c_ps"))
    nc.tensor.matmul(w8c_ps[:], lhsT=w8[:], rhs=ones[:, 0:1], start=True, stop=True)
    w8col = g.tile([E, 1], F32, tag="w8col", name=n("w8col"))
    nc.vector.tensor_copy(w8col[:], w8c_ps[:])
    bc_ps = f["gpsum"].tile([C, 1], F32, tag="gps", name=n("bc_ps"))
    nc.tensor.matmul(bc_ps[:], lhsT=bexp_sb, rhs=w8col[:], start=True, stop=True)
    b_comb = g.tile([C, 1], F32, tag="b_comb", name=n("b_comb"))
    nc.vector.tensor_copy(b_comb[:], bc_ps[:])
    return wb_sb, b_comb


def _emit_mac(nc, pools, s, wb_sb, wpsA_sb, wpsB_sb):
    """wcomb = sum_e p_e wps_e: single DVE MAC chain accumulating in bf16.
    Residual identity is pre-folded into every expert's center-tap B-half
    on the host."""
    f = pools
    wcombr = f["wcomb"].tile([128, 3, 128], BF16, tag="wcombr", name=f"wcombr{s}")
    nc.vector.tensor_scalar_mul(wcombr[:], wpsA_sb[:, 0], wb_sb[:, 0:1])
    for e in range(1, E):
        src = wpsA_sb[:, e] if e < 4 else wpsB_sb[:, e - 4]
        nc.vector.scalar_tensor_tensor(
            wcombr[:], src, wb_sb[:, e : e + 1], wcombr[:],
            op0=OP.mult, op1=OP.add,
        )
    return wcombr


def _emit_pair(nc, pools, s, p, XX, wcombr, b_comb, ob, half, ocol):
    """Conv for tile pair p: 6 (or 3) matmuls into a 2-bank PSUM tile,
    dyi-major so consecutive matmuls share lhsT. ACT stages the B half
    (+1 col) to SBUF bf16, DVE combines into the out batch buffer
    (partition half selected by batch parity)."""
    f = pools
    XX3 = XX[:, 0:FLAT].rearrange("p (r c) -> p r c", c=WP)
    tl = PAIRS[p]
    ps = f["cpsum"].tile([128, 2, 512], F32, tag="cps", name=f"cps{s}_{p}")
    for dyi in range(3):
        for t01, t in enumerate(tl):
            r0, nr = TILES[t]
            nc.tensor.matmul(
                ps[:, t01, 0 : nr * WP],
                lhsT=wcombr[:, dyi, :],
                rhs=XX3[:, r0 + dyi : r0 + dyi + nr, :],
                start=(dyi == 0),
                stop=(dyi == 2),
            )
    lo = 64 * half
    if len(tl) == 2:
        sbB = f["stage"].tile([128, 2, 390], BF16, tag="sbB", name=f"sbB{s}_{p}")
        nc.scalar.activation(sbB[0:64], ps[64:128, :, 1:391], ACTF.Copy)
        nc.vector.scalar_tensor_tensor(
            ob[lo : lo + 64, ocol : ocol + 780].rearrange("p (t c) -> p t c", c=390),
            ps[0:64, :, 0:390],
            b_comb[:],
            sbB[0:64],
            op0=OP.add,
            op1=OP.add,
        )
    else:
        ncols = TILES[tl[0]][1] * WP
        sbB = f["stage"].tile([128, 2, 390], BF16, tag="sbB", name=f"sbB{s}_{p}")
        nc.scalar.activation(
            sbB[0:64, 0, 0:ncols], ps[64:128, 0, 1 : 1 + ncols], ACTF.Copy
        )
        nc.vector.scalar_tensor_tensor(
            ob[lo : lo + 64, ocol : ocol + ncols],
            ps[0:64, 0, 0:ncols],
            b_comb[:],
            sbB[0:64, 0, 0:ncols],
            op0=OP.add,
            op1=OP.add,
        )


def build_program():
    if "nc" in _cache:
        return _cache["nc"]
    nc = bacc.Bacc("TRN2", target_bir_lowering=False, debug=False, enable_asserts=False)
    xs_ap = nc.dram_tensor("xs", [SPB, 128, FLAT], BF16, kind="ExternalInput").ap()
    wpsA_d = nc.dram_tensor("wpsA", [128, E // 2, 3, 128], BF16, kind="ExternalInput").ap()
    wpsB_d = nc.dram_tensor("wpsB", [128, E // 2, 3, 128], BF16, kind="ExternalInput").ap()
    gconst_d = nc.dram_tensor("gconst", [128, 90], F32, kind="ExternalInput").ap()
    out_ap = nc.dram_tensor("out", [SPB, 128, 3 * OBW], BF16, kind="ExternalOutput").ap()

    with tile.TileContext(nc) as tc, ExitStack() as ctx:
        pools = {
            "const": ctx.enter_context(tc.tile_pool(name="const", bufs=1)),
            "xx": ctx.enter_context(tc.tile_pool(name="xx", bufs=SPB)),
            "gate": ctx.enter_context(tc.tile_pool(name="gate", bufs=2)),
            "wcomb": ctx.enter_context(tc.tile_pool(name="wcomb", bufs=2)),
            "stage": ctx.enter_context(tc.tile_pool(name="stage", bufs=6)),
            "gpsum": ctx.enter_context(tc.tile_pool(name="gpsum", bufs=1, space="PSUM")),
            "cpsum": ctx.enter_context(tc.tile_pool(name="cpsum", bufs=3, space="PSUM")),
        }
        cp = pools["const"]
        XX0 = pools["xx"].tile([128, FLAT], BF16, tag="XX", name="XX0")
        XX1 = pools["xx"].tile([128, FLAT], BF16, tag="XX", name="XX1")
        gconst_sb = cp.tile([128, 90], F32)
        nc.gpsimd.dma_start(gconst_sb[:], gconst_d[:])
        ones = cp.tile([1, 128], F32)
        nc.gpsimd.memset(ones[:], 1.0)
        warm = cp.tile([1, 1], F32)
        nc.scalar.activation(warm[:], ones[:, 0:1], ACTF.Exp)
        wpsA_sb = cp.tile([128, E // 2, 3, 128], BF16)
        wpsB_sb = cp.tile([128, E // 2, 3, 128], BF16)
        pools["scrD"] = cp.tile([128, QC + 2], BF16, name="scrD")
        pools["scrS"] = cp.tile([128, QC + 2], BF16, name="scrS")

        wg1x2_sb = gconst_sb[:, 0:16]
        bg1_sb = gconst_sb[0:16, 16:17]
        wg2_sb = gconst_sb[0:17, 17:25]
        bexp_sb = gconst_sb[0:8, 25:89]
        consts = (wg1x2_sb, bg1_sb, wg2_sb, bexp_sb, ones)

        # ---- x loads: full-rate 128-partition contiguous chunks ----
        # SP: s0 chunks 0,1 (top-GAP feed); gps: s0 chunks 2,3 (bot-GAP
        # feed) then all of s1
        for q in (0, 1):
            nc.sync.dma_start(
                XX0[:, QC * q : QC * (q + 1)], xs_ap[0, :, QC * q : QC * (q + 1)]
            )
        nc.sync.dma_start(wpsA_sb[:], wpsA_d[:])
        nc.sync.dma_start(wpsB_sb[:], wpsB_d[:])
        for q in (2, 3):
            nc.gpsimd.dma_start(
                XX0[:, QC * q : QC * (q + 1)], xs_ap[0, :, QC * q : QC * (q + 1)]
            )
        for q in range(4):
            nc.gpsimd.dma_start(
                XX1[:, QC * q : QC * (q + 1)], xs_ap[1, :, QC * q : QC * (q + 1)]
            )
        # gate h_ext tiles (trailing 1.0 row loaded once from `ones`)
        h_ext0 = pools["gate"].tile([GH + 1, 1], F32, tag="h_ext", name="h_ext0")
        h_ext1 = pools["gate"].tile([GH + 1, 1], F32, tag="h_ext", name="h_ext1")
        nc.sync.dma_start(h_ext0[GH : GH + 1, 0:1], ones[0:1, 0:1])
        nc.sync.dma_start(h_ext1[GH : GH + 1, 0:1], ones[0:1, 0:1])

        # ---- s0 GAP: tops on ACT, bottoms on DVE ----
        part0 = pools["gate"].tile([128, 2], F32, tag="part", name="part0")
        for win in GAP_TOP:
            _emit_gap_op(nc, pools, XX0, part0, win, is_bot=False, eng="act")
        for win in GAP_BOT:
            _emit_gap_op(nc, pools, XX0, part0, win, is_bot=True, eng="dve")
        pooled0 = pools["gate"].tile([128, 1], F32, tag="pooled", name="pooled0")
        nc.vector.tensor_reduce(pooled0, part0[:], axis=AX.X, op=OP.add)
        wb0, bcomb0 = _emit_gate(nc, pools, 0, pooled0, consts, h_ext0)
        wcombr0 = _emit_mac(nc, pools, 0, wb0, wpsA_sb, wpsB_sb)

        part1 = pools["gate"].tile([128, 2], F32, tag="part", name="part1")
        s1_gap_plan = {
            1: (GAP_TOP[0], False),
            3: (GAP_TOP[1], False),
            5: (GAP_BOT[0], True),
            7: (GAP_BOT[1], True),
        }

        def s1_gap_hook(p):
            if p in s1_gap_plan:
                win, is_bot = s1_gap_plan[p]
                _emit_gap_op(nc, pools, XX1, part1, win, is_bot=is_bot, eng="dve")

        # out batching: batch b (4 pairs, 24 rows; b=5 is 8 rows) goes to
        # partition half b%2 of the [128, OBW] buffer for DMA j=b//2
        obstate = {0: [None, 0], 1: [None, 0]}  # per sample: [tile, rows_in_batch]

        def emit_sample_pairs(s, XX, wcombr, bcomb, rng, hook=None):
            for p in rng:
                batch = min(p // 4, 5)
                half = batch % 2
                j = batch // 2
                ob, orow = obstate[s]
                if ob is None:
                    if half == 0:
                        ob = pools["stage"].tile(
                            [128, OBW], BF16, tag="ob", name=f"ob{s}_{j}", bufs=3
                        )
                    else:
                        ob = obstate[s][0] if obstate[s][0] is not None else None
                    obstate[s] = [ob, 0]
                    orow = 0
                _emit_pair(nc, pools, s, p, XX, wcombr, bcomb, ob, half, orow * WP)
                orow += sum(TILES[t][1] for t in PAIRS[p])
                obstate[s][1] = orow
                brows = 24 if batch < 5 else 8
                if orow == brows:
                    if half == 1 or batch == 5:
                        nc.sync.dma_start(
                            out_ap[s, :, OBW * j : OBW * (j + 1)], ob[:]
                        )
                        obstate[s] = [None, 0]
                    else:
                        obstate[s] = [ob, 0]
                if hook is not None:
                    hook(p)

        emit_sample_pairs(0, XX0, wcombr0, bcomb0, range(GATE_SPLIT), s1_gap_hook)
        pooled1 = pools["gate"].tile([128, 1], F32, tag="pooled", name="pooled1")
        nc.vector.tensor_reduce(pooled1, part1[:], axis=AX.X, op=OP.add)
        wb1, bcomb1 = _emit_gate(nc, pools, 1, pooled1, consts, h_ext1)
        wcombr1 = _emit_mac(nc, pools, 1, wb1, wpsA_sb, wpsB_sb)
        emit_sample_pairs(0, XX0, wcombr0, bcomb0, range(GATE_SPLIT, len(PAIRS)))
        emit_sample_pairs(1, XX1, wcombr1, bcomb1, range(len(PAIRS)))

    nc.compile()
    _cache["nc"] = nc
    return nc


def host_prep(x, wg1, bg1, wg2, bg2, w_exp, b_exp):
    """Host-side layout prep + per-core sharding. Returns in_maps list."""
    x = np.asarray(x, dtype=np.float32)
    wg1 = np.asarray(wg1, dtype=np.float32)
    bg1 = np.asarray(bg1, dtype=np.float32)
    wg2 = np.asarray(wg2, dtype=np.float32)
    bg2 = np.asarray(bg2, dtype=np.float32)
    w_exp = np.asarray(w_exp, dtype=np.float32)
    b_exp = np.asarray(b_exp, dtype=np.float32)

    # x shipped as [B, 128, FLAT] bf16: rows 0:64 = zero-padded flat
    # image, rows 64:128 = the same shifted +2 elements (the conv's
    # bottom-half K copy) — both SBUF halves land in one full-rate DMA
    xpad = np.zeros((B, C, HP, WP), np.float32)
    xpad[:, :, 1 : H + 1, 1 : W + 1] = x
    flat = xpad.reshape(B, C, FLAT)
    xs = np.zeros((B, 128, FLAT), NPBF16)
    xs[:, 0:64] = flat.astype(NPBF16)
    xs[:, 64:128, 0 : FLAT - 2] = flat[:, :, 2:].astype(NPBF16)

    # wps [128, E, 3(dy), 128]: K top/bottom = taps dx 0/2 on M 0:64 (A),
    # center dx=1 on M 64:128 top (B, bottom zero). Residual identity is
    # folded into every expert's center tap (sum of probs is ~1).
    wt = np.transpose(w_exp, (2, 0, 3, 4, 1))  # [I, E, dy, dx, O]
    wps = np.zeros((128, E, 3, 128), np.float32)
    wps[0:64, :, :, 0:64] = wt[:, :, :, 0, :]
    wps[64:128, :, :, 0:64] = wt[:, :, :, 2, :]
    wps[0:64, :, :, 64:128] = wt[:, :, :, 1, :]
    ii = np.arange(64)
    wps[ii, :, 1, 64 + ii] += 1.0

    gconst = np.zeros((128, 90), np.float32)
    gconst[:, 0:16] = np.concatenate([wg1, wg1], axis=0) / (H * W)
    gconst[0:16, 16] = bg1
    gconst[0:16, 17:25] = wg2
    gconst[16, 17:25] = bg2
    gconst[0:8, 25:89] = b_exp

    shared = {
        "wpsA": np.ascontiguousarray(wps[:, 0:4]).astype(NPBF16),
        "wpsB": np.ascontiguousarray(wps[:, 4:8]).astype(NPBF16),
        "gconst": gconst,
    }
    return [
        {"xs": np.ascontiguousarray(xs[SPB * k : SPB * (k + 1)]), **shared}
        for k in range(NCORES)
    ]


def _decode_out(o):
    """[128, 3*OBW] bf16 -> [C, H, W] f32 (strip pads, reassemble batches)."""
    res = np.empty((C, H, W), np.float32)
    for b in range(6):
        j, half = b // 2, b % 2
        rows = 24 if b < 5 else 8
        blk = o[64 * half : 64 * half + 64, OBW * j : OBW * j + rows * WP]
        blk = np.asarray(blk, dtype=np.float32).reshape(C, rows, WP)
        res[:, 24 * b : 24 * b + rows, :] = blk[:, :, 0:W]
    return res


def kernel(x, wg1, bg1, wg2, bg2, w_exp, b_exp):
    nc = build_program()
    in_maps = host_prep(x, wg1, bg1, wg2, bg2, w_exp, b_exp)
    res = run_bass_kernel_spmd(nc, in_maps, list(range(NCORES)))
    out = np.empty((B, C, H, W), np.float32)
    for k in range(NCORES):
        o = np.asarray(res.results[k]["out"])
        for s in range(SPB):
            out[SPB * k + s] = _decode_out(o[s])
    return out


# revision 18
# speedup vs baseline: 1.0167x; 1.0167x over previous
"""MoE block (top-2 routed 3x3 conv experts) Trainium2 Bass kernel — v4.

Data-parallel over batch, 2 samples per core on 8 cores. The conv is
linear in the kernel, so the top-2 expert kernels are combined with the
routing probabilities first (w_comb = sum_e p_e W_e + I, the +I folding
the residual into the center tap), then one 3x3 SAME conv per sample.

Conv-as-matmul, 3 matmuls per 3-row tile: x lives zero-padded
[130x130] per channel in SBUF (bf16), partitions 0-63 = channels,
partitions 64-127 = same channels shifted +2 pixels. lhsT is
[128K, 128M]: M 0:64 ("A") = taps dx=0 (top K) / dx=2 (bottom K);
M 64:128 ("B") = center dx=1 taps (top K, bottom zero). The 3 dy reads
(390 cols each, dyi-major for LDWEIGHTS reuse) accumulate into one
PSUM bank per tile. Post-op: ACT stages psB (+1 col) to SBUF bf16 (DVE
may read only ONE PSUM operand, NCC_IBVF027), then one DVE
scalar_tensor_tensor per 2-tile pair: out = psA + b_comb + sbB.

DMA facts that shape the layout: queues are descriptor-rate-bound
(~4.3ns/desc) AND a transfer spanning only 64 partitions runs at HALF
rate. So the host ships x as [128, 16900] per sample — both SBUF
halves (padded flat + 2-px-shifted copy) prebuilt — making every x
chunk one full-rate 128-partition contiguous DMA (64B-descriptor-free).
The output is likewise written as [128, 9360]: 24-row batches packed
into alternating partition halves (host strips pads / reassembles).

Lanes: SP = s0 x chunks 0-1, wps weights, h_ext consts, out DMAs;
gpsimd = gconst, s0 x chunks 2-3, all s1 x chunks; ACT = s0-top GAP,
exp, B-half stagings; DVE = s0-bot GAP, gates, MAC chains, s1 GAP
(spread between pair combines), pair combines.
"""
import numpy as np
from contextlib import ExitStack

import ml_dtypes

import concourse.bass as bass
import concourse.tile as tile
from concourse import bacc, mybir
from concourse.bass_utils import run_bass_kernel_spmd

F32 = mybir.dt.float32
BF16 = mybir.dt.bfloat16
AX = mybir.AxisListType
OP = mybir.AluOpType
ACTF = mybir.ActivationFunctionType

B, C, H, W, E, GH = 16, 64, 128, 128, 8, 16
NCORES = 8
SPB = B // NCORES          # samples per core
HP, WP = H + 2, W + 2      # 130
FLAT = HP * WP             # 16900
QC = FLAT // 4             # x-load chunk size (4225 flat elements)
OBW = 24 * WP              # out batch region width (3120)
TILES = [(3 * t, 3) for t in range(42)] + [(126, 2)]
PAIRS = [(2 * p, 2 * p + 1) for p in range(21)] + [(42,)]
GATE_SPLIT = 13            # emit s1 gate work after this many s0 pairs

NPBF16 = ml_dtypes.bfloat16

_cache = {}

# GAP windows over the flat layout (pad zeros included): top covers x
# rows 0..63 (+ the first elem of x row 64 that the shifted bottom copy
# misses), bottom covers x rows 64..127.
GAP_TOP = [(0, QC, 0), (QC, 2 * QC + 2, 1)]
GAP_BOT = [(2 * QC, 3 * QC, 0), (3 * QC, FLAT, 1)]


def _emit_gap_op(nc, pools, XX, part, win, is_bot, eng):
    a, b, slot = win
    lo, hi = (64, 128) if is_bot else (0, 64)
    src = XX[lo:hi, a:b]
    dst = pools["scrS" if eng == "act" else "scrD"][lo:hi]
    acc = part[lo:hi, slot : slot + 1]
    if eng == "act":
        nc.scalar.activation(dst[:, 0 : b - a], src, ACTF.Copy, accum_out=acc)
    else:
        nc.vector.tensor_scalar(
            dst[:, 0 : b - a], src, 0.0, 0.0, OP.add, OP.add, accum_out=acc
        )


def _emit_gate(nc, pools, s, pooled, consts, h_ext):
    """Gate MLP + softmax + top-2 for one sample (all f32).

    exp-without-max-sub (logits are small); folds the top-2 mask and
    renormalization: w8 = (u>=m2)*u / (sum((u>=m2)*u) + sum(u)*1e-8).
    Returns (wb_sb [128,E] f32 per-partition probs, b_comb [C,1]).
    """
    f = pools
    g = f["gate"]
    wg1x2_sb, bg1_sb, wg2_sb, bexp_sb, ones = consts
    n = lambda base: f"{base}{s}"

    h_ps = f["gpsum"].tile([GH, 1], F32, tag="gps", name=n("h_ps"))
    nc.tensor.matmul(h_ps[:], lhsT=wg1x2_sb, rhs=pooled[:], start=True, stop=True)
    nc.vector.tensor_scalar(h_ext[0:GH, :], h_ps[:], bg1_sb, 0.0, OP.add, OP.max)

    lg_ps = f["gpsum"].tile([1, E], F32, tag="gps", name=n("lg_ps"))
    nc.tensor.matmul(lg_ps[:], lhsT=h_ext[:], rhs=wg2_sb, start=True, stop=True)

    u = g.tile([1, E], F32, tag="u", name=n("u"))
    nc.scalar.activation(u[:], lg_ps[:], ACTF.Exp)
    usum = g.tile([1, 1], F32, tag="usum", name=n("usum"))
    nc.vector.tensor_reduce(usum[:], u[:], axis=AX.X, op=OP.add)
    m1p = g.tile([1, 1], F32, tag="m1p", name=n("m1p"))
    nc.vector.tensor_reduce(m1p[:], u[:], axis=AX.X, op=OP.max)
    pm = g.tile([1, E], F32, tag="pm", name=n("pm"))
    nc.vector.scalar_tensor_tensor(pm[:], u[:], m1p[:], u[:], op0=OP.is_lt, op1=OP.mult)
    m2 = g.tile([1, 1], F32, tag="m2", name=n("m2"))
    nc.vector.tensor_reduce(m2[:], pm[:], axis=AX.X, op=OP.max)
    spv = g.tile([1, E], F32, tag="spv", name=n("spv"))
    nc.vector.scalar_tensor_tensor(spv[:], u[:], m2[:], u[:], op0=OP.is_ge, op1=OP.mult)
    dsum = g.tile([1, 1], F32, tag="dsum", name=n("dsum"))
    nc.vector.tensor_reduce(dsum[:], spv[:], axis=AX.X, op=OP.add)
    dd = g.tile([1, 1], F32, tag="dd", name=n("dd"))
    nc.vector.scalar_tensor_tensor(dd[:], usum[:], 1e-8, dsum[:], op0=OP.mult, op1=OP.add)
    rr = g.tile([1, 1], F32, tag="rr", name=n("rr"))
    nc.vector.reciprocal(rr[:], dd[:])
    w8 = g.tile([1, E], F32, tag="w8", name=n("w8"))
    nc.vector.tensor_scalar_mul(w8[:], spv[:], rr[:])

    # broadcast w8 down all 128 partitions, then stage to SBUF for MACs
    wb_ps = f["gpsum"].tile([128, E], F32, tag="gps", name=n("wb_ps"))
    nc.tensor.matmul(wb_ps[:], lhsT=ones[:], rhs=w8[:], start=True, stop=True)
    wb_sb = g.tile([128, E], F32, tag="wb_sb", name=n("wb_sb"))
    nc.vector.tensor_copy(wb_sb[:], wb_ps[:])

    # combined bias: b_comb = b_exp^T @ w8^T
    w8c_ps = f["gpsum"].tile([E, 1], F32, tag="gps", name=n("w8c_ps"))
    nc.tensor.matmul(w8c_ps[:], lhsT=w8[:], rhs=ones[:, 0:1], start=True, stop=True)
    w8col = g.tile([E, 1], F32, tag="w8col", name=n("w8col"))
    nc.vector.tensor_copy(w8col[:], w8c_ps[:])
    bc_ps = f["gpsum"].tile([C, 1], F32, tag="gps", name=n("bc_ps"))
    nc.tensor.matmul(bc_ps[:], lhsT=bexp_sb, rhs=w8col[:], start=True, stop=True)
    b_comb = g.tile([C, 1], F32, tag="b_comb", name=n("b_comb"))
    nc.vector.tensor_copy(b_comb[:], bc_ps[:])
    return wb_sb, b_comb


def _emit_mac(nc, pools, s, wb_sb, wpsA_sb, wpsB_sb):
    """wcomb = sum_e p_e wps_e: single DVE MAC chain accumulating in bf16.
    Residual identity is pre-folded into every expert's center-tap B-half
    on the host."""
    f = pools
    wcombr = f["wcomb"].tile([128, 3, 128], BF16, tag="wcombr", name=f"wcombr{s}")
    nc.vector.tensor_scalar_mul(wcombr[:], wpsA_sb[:, 0], wb_sb[:, 0:1])
    for e in range(1, E):
        src = wpsA_sb[:, e] if e < 4 else wpsB_sb[:, e - 4]
        nc.vector.scalar_tensor_tensor(
            wcombr[:], src, wb_sb[:, e : e + 1], wcombr[:],
            op0=OP.mult, op1=OP.add,
        )
    return wcombr


def _emit_pair(nc, pools, s, p, XX, wcombr, b_comb, ob, half, ocol):
    """Conv for tile pair p: 6 (or 3) matmuls into a 2-bank PSUM tile,
    dyi-major so consecutive matmuls share lhsT. ACT stages the B half
    (+1 col) to SBUF bf16, DVE combines into the out batch buffer
    (partition half selected by batch parity)."""
    f = pools
    XX3 = XX[:, 0:FLAT].rearrange("p (r c) -> p r c", c=WP)
    tl = PAIRS[p]
    ps = f["cpsum"].tile([128, 2, 512], F32, tag="cps", name=f"cps{s}_{p}")
    for dyi in range(3):
        for t01, t in enumerate(tl):
            r0, nr = TILES[t]
            nc.tensor.matmul(
                ps[:, t01, 0 : nr * WP],
                lhsT=wcombr[:, dyi, :],
                rhs=XX3[:, r0 + dyi : r0 + dyi + nr, :],
                start=(dyi == 0),
                stop=(dyi == 2),
            )
    lo = 64 * half
    if len(tl) == 2:
        sbB = f["stage"].tile([128, 2, 390], BF16, tag="sbB", name=f"sbB{s}_{p}")
        nc.scalar.activation(sbB[0:64], ps[64:128, :, 1:391], ACTF.Copy)
        nc.vector.scalar_tensor_tensor(
            ob[lo : lo + 64, ocol : ocol + 780].rearrange("p (t c) -> p t c", c=390),
            ps[0:64, :, 0:390],
            b_comb[:],
            sbB[0:64],
            op0=OP.add,
            op1=OP.add,
        )
    else:
        ncols = TILES[tl[0]][1] * WP
        sbB = f["stage"].tile([128, 2, 390], BF16, tag="sbB", name=f"sbB{s}_{p}")
        nc.scalar.activation(
            sbB[0:64, 0, 0:ncols], ps[64:128, 0, 1 : 1 + ncols], ACTF.Copy
        )
        nc.vector.scalar_tensor_tensor(
            ob[lo : lo + 64, ocol : ocol + ncols],
            ps[0:64, 0, 0:ncols],
            b_comb[:],
            sbB[0:64, 0, 0:ncols],
            op0=OP.add,
            op1=OP.add,
        )


def build_program():
    if "nc" in _cache:
        return _cache["nc"]
    nc = bacc.Bacc("TRN2", target_bir_lowering=False, debug=False, enable_asserts=False)
    xs_ap = nc.dram_tensor("xs", [SPB, 128, FLAT], BF16, kind="ExternalInput").ap()
    wpsA_d = nc.dram_tensor("wpsA", [128, E // 2, 3, 128], BF16, kind="ExternalInput").ap()
    wpsB_d = nc.dram_tensor("wpsB", [128, E // 2, 3, 128], BF16, kind="ExternalInput").ap()
    gconst_d = nc.dram_tensor("gconst", [128, 90], F32, kind="ExternalInput").ap()
    out_ap = nc.dram_tensor("out", [SPB, 128, 3 * OBW], BF16, kind="ExternalOutput").ap()

    with tile.TileContext(nc) as tc, ExitStack() as ctx:
        pools = {
            "const": ctx.enter_context(tc.tile_pool(name="const", bufs=1)),
            "xx": ctx.enter_context(tc.tile_pool(name="xx", bufs=SPB)),
            "gate": ctx.enter_context(tc.tile_pool(name="gate", bufs=2)),
            "wcomb": ctx.enter_context(tc.tile_pool(name="wcomb", bufs=2)),
            "stage": ctx.enter_context(tc.tile_pool(name="stage", bufs=6)),
            "gpsum": ctx.enter_context(tc.tile_pool(name="gpsum", bufs=1, space="PSUM")),
            "cpsum": ctx.enter_context(tc.tile_pool(name="cpsum", bufs=3, space="PSUM")),
        }
        cp = pools["const"]
        XX0 = pools["xx"].tile([128, FLAT], BF16, tag="XX", name="XX0")
        XX1 = pools["xx"].tile([128, FLAT], BF16, tag="XX", name="XX1")
        gconst_sb = cp.tile([128, 90], F32)
        nc.gpsimd.dma_start(gconst_sb[:], gconst_d[:])
        ones = cp.tile([1, 128], F32)
        nc.gpsimd.memset(ones[:], 1.0)
        warm = cp.tile([1, 1], F32)
        nc.scalar.activation(warm[:], ones[:, 0:1], ACTF.Exp)
        wpsA_sb = cp.tile([128, E // 2, 3, 128], BF16)
        wpsB_sb = cp.tile([128, E // 2, 3, 128], BF16)
        pools["scrD"] = cp.tile([128, QC + 2], BF16, name="scrD")
        pools["scrS"] = cp.tile([128, QC + 2], BF16, name="scrS")

        wg1x2_sb = gconst_sb[:, 0:16]
        bg1_sb = gconst_sb[0:16, 16:17]
        wg2_sb = gconst_sb[0:17, 17:25]
        bexp_sb = gconst_sb[0:8, 25:89]
        consts = (wg1x2_sb, bg1_sb, wg2_sb, bexp_sb, ones)

        # ---- x loads: full-rate 128-partition contiguous chunks ----
        # SP: s0 chunks 0,1 (top-GAP feed); gps: s0 chunks 2,3 (bot-GAP
        # feed) then all of s1
        for q in (0, 1):
            nc.sync.dma_start(
                XX0[:, QC * q : QC * (q + 1)], xs_ap[0, :, QC * q : QC * (q + 1)]
            )
        nc.sync.dma_start(wpsA_sb[:], wpsA_d[:])
        nc.sync.dma_start(wpsB_sb[:], wpsB_d[:])
        for q in (2, 3):
            nc.scalar.dma_start(
                XX0[:, QC * q : QC * (q + 1)], xs_ap[0, :, QC * q : QC * (q + 1)]
            )
        for q in range(4):
            nc.scalar.dma_start(
                XX1[:, QC * q : QC * (q + 1)], xs_ap[1, :, QC * q : QC * (q + 1)]
            )
        # gate h_ext tiles (trailing 1.0 row loaded once from `ones`)
        h_ext0 = pools["gate"].tile([GH + 1, 1], F32, tag="h_ext", name="h_ext0")
        h_ext1 = pools["gate"].tile([GH + 1, 1], F32, tag="h_ext", name="h_ext1")
        nc.sync.dma_start(h_ext0[GH : GH + 1, 0:1], ones[0:1, 0:1])
        nc.sync.dma_start(h_ext1[GH : GH + 1, 0:1], ones[0:1, 0:1])

        # ---- s0 GAP: tops on ACT, bottoms on DVE ----
        part0 = pools["gate"].tile([128, 2], F32, tag="part", name="part0")
        for win in GAP_TOP:
            _emit_gap_op(nc, pools, XX0, part0, win, is_bot=False, eng="act")
        for win in GAP_BOT:
            _emit_gap_op(nc, pools, XX0, part0, win, is_bot=True, eng="dve")
        pooled0 = pools["gate"].tile([128, 1], F32, tag="pooled", name="pooled0")
        nc.vector.tensor_reduce(pooled0, part0[:], axis=AX.X, op=OP.add)
        wb0, bcomb0 = _emit_gate(nc, pools, 0, pooled0, consts, h_ext0)
        wcombr0 = _emit_mac(nc, pools, 0, wb0, wpsA_sb, wpsB_sb)

        part1 = pools["gate"].tile([128, 2], F32, tag="part", name="part1")
        s1_gap_plan = {
            3: (GAP_TOP[0], False, "dve"),
            6: (GAP_TOP[1], False, "act"),
            9: (GAP_BOT[0], True, "dve"),
            12: (GAP_BOT[1], True, "act"),
        }

        def s1_gap_hook(p):
            if p in s1_gap_plan:
                win, is_bot, eng = s1_gap_plan[p]
                _emit_gap_op(nc, pools, XX1, part1, win, is_bot=is_bot, eng=eng)

        # out batching: batch b (4 pairs, 24 rows; b=5 is 8 rows) goes to
        # partition half b%2 of the [128, OBW] buffer for DMA j=b//2
        obstate = {0: [None, 0], 1: [None, 0]}  # per sample: [tile, rows_in_batch]

        def emit_sample_pairs(s, XX, wcombr, bcomb, rng, hook=None):
            for p in rng:
                batch = min(p // 4, 5)
                half = batch % 2
                j = batch // 2
                ob, orow = obstate[s]
                if ob is None:
                    if half == 0:
                        ob = pools["stage"].tile(
                            [128, OBW], BF16, tag="ob", name=f"ob{s}_{j}", bufs=3
                        )
                    else:
                        ob = obstate[s][0] if obstate[s][0] is not None else None
                    obstate[s] = [ob, 0]
                    orow = 0
                _emit_pair(nc, pools, s, p, XX, wcombr, bcomb, ob, half, orow * WP)
                orow += sum(TILES[t][1] for t in PAIRS[p])
                obstate[s][1] = orow
                brows = 24 if batch < 5 else 8
                if orow == brows:
                    if half == 1 or batch == 5:
                        nc.sync.dma_start(
                            out_ap[s, :, OBW * j : OBW * (j + 1)], ob[:]
                        )
                        obstate[s] = [None, 0]
                    else:
                        obstate[s] = [ob, 0]
                if hook is not None:
                    hook(p)

        emit_sample_pairs(0, XX0, wcombr0, bcomb0, range(GATE_SPLIT), s1_gap_hook)
        pooled1 = pools["gate"].tile([128, 1], F32, tag="pooled", name="pooled1")
        nc.vector.tensor_reduce(pooled1, part1[:], axis=AX.X, op=OP.add)
        wb1, bcomb1 = _emit_gate(nc, pools, 1, pooled1, consts, h_ext1)
        wcombr1 = _emit_mac(nc, pools, 1, wb1, wpsA_sb, wpsB_sb)
        emit_sample_pairs(0, XX0, wcombr0, bcomb0, range(GATE_SPLIT, len(PAIRS)))
        emit_sample_pairs(1, XX1, wcombr1, bcomb1, range(len(PAIRS)))

    nc.compile()
    _cache["nc"] = nc
    return nc


def host_prep(x, wg1, bg1, wg2, bg2, w_exp, b_exp):
    """Host-side layout prep + per-core sharding. Returns in_maps list."""
    x = np.asarray(x, dtype=np.float32)
    wg1 = np.asarray(wg1, dtype=np.float32)
    bg1 = np.asarray(bg1, dtype=np.float32)
    wg2 = np.asarray(wg2, dtype=np.float32)
    bg2 = np.asarray(bg2, dtype=np.float32)
    w_exp = np.asarray(w_exp, dtype=np.float32)
    b_exp = np.asarray(b_exp, dtype=np.float32)

    # x shipped as [B, 128, FLAT] bf16: rows 0:64 = zero-padded flat
    # image, rows 64:128 = the same shifted +2 elements (the conv's
    # bottom-half K copy) — both SBUF halves land in one full-rate DMA
    xpad = np.zeros((B, C, HP, WP), np.float32)
    xpad[:, :, 1 : H + 1, 1 : W + 1] = x
    flat = xpad.reshape(B, C, FLAT)
    xs = np.zeros((B, 128, FLAT), NPBF16)
    xs[:, 0:64] = flat.astype(NPBF16)
    xs[:, 64:128, 0 : FLAT - 2] = flat[:, :, 2:].astype(NPBF16)

    # wps [128, E, 3(dy), 128]: K top/bottom = taps dx 0/2 on M 0:64 (A),
    # center dx=1 on M 64:128 top (B, bottom zero). Residual identity is
    # folded into every expert's center tap (sum of probs is ~1).
    wt = np.transpose(w_exp, (2, 0, 3, 4, 1))  # [I, E, dy, dx, O]
    wps = np.zeros((128, E, 3, 128), np.float32)
    wps[0:64, :, :, 0:64] = wt[:, :, :, 0, :]
    wps[64:128, :, :, 0:64] = wt[:, :, :, 2, :]
    wps[0:64, :, :, 64:128] = wt[:, :, :, 1, :]
    ii = np.arange(64)
    wps[ii, :, 1, 64 + ii] += 1.0

    gconst = np.zeros((128, 90), np.float32)
    gconst[:, 0:16] = np.concatenate([wg1, wg1], axis=0) / (H * W)
    gconst[0:16, 16] = bg1
    gconst[0:16, 17:25] = wg2
    gconst[16, 17:25] = bg2
    gconst[0:8, 25:89] = b_exp

    shared = {
        "wpsA": np.ascontiguousarray(wps[:, 0:4]).astype(NPBF16),
        "wpsB": np.ascontiguousarray(wps[:, 4:8]).astype(NPBF16),
        "gconst": gconst,
    }
    return [
        {"xs": np.ascontiguousarray(xs[SPB * k : SPB * (k + 1)]), **shared}
        for k in range(NCORES)
    ]


def _decode_out(o):
    """[128, 3*OBW] bf16 -> [C, H, W] f32 (strip pads, reassemble batches)."""
    res = np.empty((C, H, W), np.float32)
    for b in range(6):
        j, half = b // 2, b % 2
        rows = 24 if b < 5 else 8
        blk = o[64 * half : 64 * half + 64, OBW * j : OBW * j + rows * WP]
        blk = np.asarray(blk, dtype=np.float32).reshape(C, rows, WP)
        res[:, 24 * b : 24 * b + rows, :] = blk[:, :, 0:W]
    return res


def kernel(x, wg1, bg1, wg2, bg2, w_exp, b_exp):
    nc = build_program()
    in_maps = host_prep(x, wg1, bg1, wg2, bg2, w_exp, b_exp)
    res = run_bass_kernel_spmd(nc, in_maps, list(range(NCORES)))
    out = np.empty((B, C, H, W), np.float32)
    for k in range(NCORES):
        o = np.asarray(res.results[k]["out"])
        for s in range(SPB):
            out[SPB * k + s] = _decode_out(o[s])
    return out


# revision 19
# speedup vs baseline: 1.0563x; 1.0389x over previous
"""MoE block (top-2 routed 3x3 conv experts) Trainium2 Bass kernel — v4.

Data-parallel over batch, 2 samples per core on 8 cores. The conv is
linear in the kernel, so the top-2 expert kernels are combined with the
routing probabilities first (w_comb = sum_e p_e W_e + I, the +I folding
the residual into the center tap), then one 3x3 SAME conv per sample.

Conv-as-matmul, 3 matmuls per 3-row tile: x lives zero-padded
[130x130] per channel in SBUF (bf16), partitions 0-63 = channels,
partitions 64-127 = same channels shifted +2 pixels. lhsT is
[128K, 128M]: M 0:64 ("A") = taps dx=0 (top K) / dx=2 (bottom K);
M 64:128 ("B") = center dx=1 taps (top K, bottom zero). The 3 dy reads
(390 cols each, dyi-major for LDWEIGHTS reuse) accumulate into one
PSUM bank per tile. Post-op: ACT stages psB (+1 col) to SBUF bf16 (DVE
may read only ONE PSUM operand, NCC_IBVF027), then one DVE
scalar_tensor_tensor per 2-tile pair: out = psA + b_comb + sbB.

DMA facts that shape the layout: queues are descriptor-rate-bound
(~4.3ns/desc) AND a transfer spanning only 64 partitions runs at HALF
rate. So the host ships x as [128, 16900] per sample — both SBUF
halves (padded flat + 2-px-shifted copy) prebuilt — making every x
chunk one full-rate 128-partition contiguous DMA (64B-descriptor-free).
The output is likewise written as [128, 9360]: 24-row batches packed
into alternating partition halves (host strips pads / reassembles).

Lanes: SP = s0 x chunks 0-1, wps weights, h_ext consts, out DMAs;
gpsimd = gconst, s0 x chunks 2-3, all s1 x chunks; ACT = s0-top GAP,
exp, B-half stagings; DVE = s0-bot GAP, gates, MAC chains, s1 GAP
(spread between pair combines), pair combines.
"""
import numpy as np
from contextlib import ExitStack

import ml_dtypes

import concourse.bass as bass
import concourse.tile as tile
from concourse import bacc, mybir
from concourse.bass_utils import run_bass_kernel_spmd

F32 = mybir.dt.float32
BF16 = mybir.dt.bfloat16
AX = mybir.AxisListType
OP = mybir.AluOpType
ACTF = mybir.ActivationFunctionType

B, C, H, W, E, GH = 16, 64, 128, 128, 8, 16
NCORES = 8
SPB = B // NCORES          # samples per core
HP, WP = H + 2, W + 2      # 130
FLAT = HP * WP             # 16900
QC = FLAT // 4             # x-load chunk size (4225 flat elements)
OBW = 24 * WP              # out batch region width (3120)
TILES = [(3 * t, 3) for t in range(42)] + [(126, 2)]
PAIRS = [(2 * p, 2 * p + 1) for p in range(21)] + [(42,)]
GATE_SPLIT = 14            # emit s1 gate work after this many s0 pairs

NPBF16 = ml_dtypes.bfloat16

_cache = {}

# GAP windows over the flat layout (pad zeros included): top covers x
# rows 0..63 (+ the first elem of x row 64 that the shifted bottom copy
# misses), bottom covers x rows 64..127.
GAP_TOP = [(0, QC, 0), (QC, 2 * QC + 2, 1)]
GAP_BOT = [(2 * QC, 3 * QC, 0), (3 * QC, FLAT, 1)]


def _emit_gap_op(nc, pools, XX, part, win, is_bot, eng):
    a, b, slot = win
    lo, hi = (64, 128) if is_bot else (0, 64)
    src = XX[lo:hi, a:b]
    dst = pools["scrS" if eng == "act" else "scrD"][lo:hi]
    acc = part[lo:hi, slot : slot + 1]
    if eng == "act":
        nc.scalar.activation(dst[:, 0 : b - a], src, ACTF.Copy, accum_out=acc)
    else:
        nc.vector.tensor_scalar(
            dst[:, 0 : b - a], src, 0.0, 0.0, OP.add, OP.add, accum_out=acc
        )


def _emit_gate(nc, pools, s, pooled, consts, h_ext):
    """Gate MLP + softmax + top-2 for one sample (all f32).

    exp-without-max-sub (logits are small); folds the top-2 mask and
    renormalization: w8 = (u>=m2)*u / (sum((u>=m2)*u) + sum(u)*1e-8).
    Returns (wb_sb [128,E] f32 per-partition probs, b_comb [C,1]).
    """
    f = pools
    g = f["gate"]
    wg1x2_sb, bg1_sb, wg2_sb, bexp_sb, ones = consts
    n = lambda base: f"{base}{s}"

    h_ps = f["gpsum"].tile([GH, 1], F32, tag="gps", name=n("h_ps"))
    nc.tensor.matmul(h_ps[:], lhsT=wg1x2_sb, rhs=pooled[:], start=True, stop=True)
    nc.vector.tensor_scalar(h_ext[0:GH, :], h_ps[:], bg1_sb, 0.0, OP.add, OP.max)

    lg_ps = f["gpsum"].tile([1, E], F32, tag="gps", name=n("lg_ps"))
    nc.tensor.matmul(lg_ps[:], lhsT=h_ext[:], rhs=wg2_sb, start=True, stop=True)

    u = g.tile([1, E], F32, tag="u", name=n("u"))
    nc.scalar.activation(u[:], lg_ps[:], ACTF.Exp)
    usum = g.tile([1, 1], F32, tag="usum", name=n("usum"))
    nc.vector.tensor_reduce(usum[:], u[:], axis=AX.X, op=OP.add)
    m1p = g.tile([1, 1], F32, tag="m1p", name=n("m1p"))
    nc.vector.tensor_reduce(m1p[:], u[:], axis=AX.X, op=OP.max)
    pm = g.tile([1, E], F32, tag="pm", name=n("pm"))
    nc.vector.scalar_tensor_tensor(pm[:], u[:], m1p[:], u[:], op0=OP.is_lt, op1=OP.mult)
    m2 = g.tile([1, 1], F32, tag="m2", name=n("m2"))
    nc.vector.tensor_reduce(m2[:], pm[:], axis=AX.X, op=OP.max)
    spv = g.tile([1, E], F32, tag="spv", name=n("spv"))
    nc.vector.scalar_tensor_tensor(spv[:], u[:], m2[:], u[:], op0=OP.is_ge, op1=OP.mult)
    dsum = g.tile([1, 1], F32, tag="dsum", name=n("dsum"))
    nc.vector.tensor_reduce(dsum[:], spv[:], axis=AX.X, op=OP.add)
    dd = g.tile([1, 1], F32, tag="dd", name=n("dd"))
    nc.vector.scalar_tensor_tensor(dd[:], usum[:], 1e-8, dsum[:], op0=OP.mult, op1=OP.add)
    rr = g.tile([1, 1], F32, tag="rr", name=n("rr"))
    nc.vector.reciprocal(rr[:], dd[:])
    w8 = g.tile([1, E], F32, tag="w8", name=n("w8"))
    nc.vector.tensor_scalar_mul(w8[:], spv[:], rr[:])

    # broadcast w8 down all 128 partitions, then stage to SBUF for MACs
    wb_ps = f["gpsum"].tile([128, E], F32, tag="gps", name=n("wb_ps"))
    nc.tensor.matmul(wb_ps[:], lhsT=ones[:], rhs=w8[:], start=True, stop=True)
    wb_sb = g.tile([128, E], F32, tag="wb_sb", name=n("wb_sb"))
    nc.vector.tensor_copy(wb_sb[:], wb_ps[:])

    # combined bias: b_comb = b_exp^T @ w8^T
    w8c_ps = f["gpsum"].tile([E, 1], F32, tag="gps", name=n("w8c_ps"))
    nc.tensor.matmul(w8c_ps[:], lhsT=w8[:], rhs=ones[:, 0:1], start=True, stop=True)
    w8col = g.tile([E, 1], F32, tag="w8col", name=n("w8col"))
    nc.vector.tensor_copy(w8col[:], w8c_ps[:])
    bc_ps = f["gpsum"].tile([C, 1], F32, tag="gps", name=n("bc_ps"))
    nc.tensor.matmul(bc_ps[:], lhsT=bexp_sb, rhs=w8col[:], start=True, stop=True)
    b_comb = g.tile([C, 1], F32, tag="b_comb", name=n("b_comb"))
    nc.vector.tensor_copy(b_comb[:], bc_ps[:])
    return wb_sb, b_comb


def _emit_mac(nc, pools, s, wb_sb, wpsA_sb, wpsB_sb):
    """wcomb = sum_e p_e wps_e: single DVE MAC chain accumulating in bf16.
    Residual identity is pre-folded into every expert's center-tap B-half
    on the host."""
    f = pools
    wcombr = f["wcomb"].tile([128, 3, 128], BF16, tag="wcombr", name=f"wcombr{s}")
    nc.vector.tensor_scalar_mul(wcombr[:], wpsA_sb[:, 0], wb_sb[:, 0:1])
    for e in range(1, E):
        src = wpsA_sb[:, e] if e < 4 else wpsB_sb[:, e - 4]
        nc.vector.scalar_tensor_tensor(
            wcombr[:], src, wb_sb[:, e : e + 1], wcombr[:],
            op0=OP.mult, op1=OP.add,
        )
    return wcombr


def _emit_pair(nc, pools, s, p, XX, wcombr, b_comb, ob, half, ocol):
    """Conv for tile pair p: 6 (or 3) matmuls into a 2-bank PSUM tile,
    dyi-major so consecutive matmuls share lhsT. ACT stages the B half
    (+1 col) to SBUF bf16, DVE combines into the out batch buffer
    (partition half selected by batch parity)."""
    f = pools
    XX3 = XX[:, 0:FLAT].rearrange("p (r c) -> p r c", c=WP)
    tl = PAIRS[p]
    ps = f["cpsum"].tile([128, 2, 512], F32, tag="cps", name=f"cps{s}_{p}")
    for dyi in range(3):
        for t01, t in enumerate(tl):
            r0, nr = TILES[t]
            nc.tensor.matmul(
                ps[:, t01, 0 : nr * WP],
                lhsT=wcombr[:, dyi, :],
                rhs=XX3[:, r0 + dyi : r0 + dyi + nr, :],
                start=(dyi == 0),
                stop=(dyi == 2),
            )
    lo = 64 * half
    if len(tl) == 2:
        sbB = f["stage"].tile([128, 2, 390], BF16, tag="sbB", name=f"sbB{s}_{p}")
        nc.scalar.activation(sbB[0:64], ps[64:128, :, 1:391], ACTF.Copy)
        nc.vector.scalar_tensor_tensor(
            ob[lo : lo + 64, ocol : ocol + 780].rearrange("p (t c) -> p t c", c=390),
            ps[0:64, :, 0:390],
            b_comb[:],
            sbB[0:64],
            op0=OP.add,
            op1=OP.add,
        )
    else:
        ncols = TILES[tl[0]][1] * WP
        sbB = f["stage"].tile([128, 2, 390], BF16, tag="sbB", name=f"sbB{s}_{p}")
        nc.scalar.activation(
            sbB[0:64, 0, 0:ncols], ps[64:128, 0, 1 : 1 + ncols], ACTF.Copy
        )
        nc.vector.scalar_tensor_tensor(
            ob[lo : lo + 64, ocol : ocol + ncols],
            ps[0:64, 0, 0:ncols],
            b_comb[:],
            sbB[0:64, 0, 0:ncols],
            op0=OP.add,
            op1=OP.add,
        )


def build_program():
    if "nc" in _cache:
        return _cache["nc"]
    nc = bacc.Bacc("TRN2", target_bir_lowering=False, debug=False, enable_asserts=False)
    xs_ap = nc.dram_tensor("xs", [SPB, 128, FLAT], BF16, kind="ExternalInput").ap()
    wpsA_d = nc.dram_tensor("wpsA", [128, E // 2, 3, 128], BF16, kind="ExternalInput").ap()
    wpsB_d = nc.dram_tensor("wpsB", [128, E // 2, 3, 128], BF16, kind="ExternalInput").ap()
    gconst_d = nc.dram_tensor("gconst", [128, 90], F32, kind="ExternalInput").ap()
    out_ap = nc.dram_tensor("out", [SPB, 128, 3 * OBW], BF16, kind="ExternalOutput").ap()

    with tile.TileContext(nc) as tc, ExitStack() as ctx:
        pools = {
            "const": ctx.enter_context(tc.tile_pool(name="const", bufs=1)),
            "xx": ctx.enter_context(tc.tile_pool(name="xx", bufs=SPB)),
            "gate": ctx.enter_context(tc.tile_pool(name="gate", bufs=2)),
            "wcomb": ctx.enter_context(tc.tile_pool(name="wcomb", bufs=2)),
            "stage": ctx.enter_context(tc.tile_pool(name="stage", bufs=6)),
            "gpsum": ctx.enter_context(tc.tile_pool(name="gpsum", bufs=1, space="PSUM")),
            "cpsum": ctx.enter_context(tc.tile_pool(name="cpsum", bufs=3, space="PSUM")),
        }
        cp = pools["const"]
        XX0 = pools["xx"].tile([128, FLAT], BF16, tag="XX", name="XX0")
        XX1 = pools["xx"].tile([128, FLAT], BF16, tag="XX", name="XX1")
        gconst_sb = cp.tile([128, 90], F32)
        nc.gpsimd.dma_start(gconst_sb[:], gconst_d[:])
        ones = cp.tile([1, 128], F32)
        nc.gpsimd.memset(ones[:], 1.0)
        warm = cp.tile([1, 1], F32)
        nc.scalar.activation(warm[:], ones[:, 0:1], ACTF.Exp)
        wpsA_sb = cp.tile([128, E // 2, 3, 128], BF16)
        wpsB_sb = cp.tile([128, E // 2, 3, 128], BF16)
        pools["scrD"] = cp.tile([128, QC + 2], BF16, name="scrD")
        pools["scrS"] = cp.tile([128, QC + 2], BF16, name="scrS")

        wg1x2_sb = gconst_sb[:, 0:16]
        bg1_sb = gconst_sb[0:16, 16:17]
        wg2_sb = gconst_sb[0:17, 17:25]
        bexp_sb = gconst_sb[0:8, 25:89]
        consts = (wg1x2_sb, bg1_sb, wg2_sb, bexp_sb, ones)

        # ---- x loads: full-rate 128-partition contiguous chunks ----
        # SP and gpsimd are pure DMA lanes (ACT stays compute-only so GAP
        # ops and stagings never queue behind transfers): SP = s0 chunks
        # 0,1 + weights + all of s1; gps = s0 chunks 2,3 + out DMAs
        for q in (0, 1):
            nc.sync.dma_start(
                XX0[:, QC * q : QC * (q + 1)], xs_ap[0, :, QC * q : QC * (q + 1)]
            )
        for q in (2, 3):
            nc.gpsimd.dma_start(
                XX0[:, QC * q : QC * (q + 1)], xs_ap[0, :, QC * q : QC * (q + 1)]
            )
        nc.sync.dma_start(wpsA_sb[:], wpsA_d[:])
        nc.sync.dma_start(wpsB_sb[:], wpsB_d[:])
        # gate h_ext tiles (trailing 1.0 row loaded once from `ones`)
        h_ext0 = pools["gate"].tile([GH + 1, 1], F32, tag="h_ext", name="h_ext0")
        h_ext1 = pools["gate"].tile([GH + 1, 1], F32, tag="h_ext", name="h_ext1")
        nc.sync.dma_start(h_ext0[GH : GH + 1, 0:1], ones[0:1, 0:1])
        nc.sync.dma_start(h_ext1[GH : GH + 1, 0:1], ones[0:1, 0:1])
        for q in range(4):
            nc.sync.dma_start(
                XX1[:, QC * q : QC * (q + 1)], xs_ap[1, :, QC * q : QC * (q + 1)]
            )

        # ---- s0 GAP: tops on ACT, bottoms on DVE ----
        part0 = pools["gate"].tile([128, 2], F32, tag="part", name="part0")
        for win in GAP_TOP:
            _emit_gap_op(nc, pools, XX0, part0, win, is_bot=False, eng="act")
        for win in GAP_BOT:
            _emit_gap_op(nc, pools, XX0, part0, win, is_bot=True, eng="dve")
        pooled0 = pools["gate"].tile([128, 1], F32, tag="pooled", name="pooled0")
        nc.vector.tensor_reduce(pooled0, part0[:], axis=AX.X, op=OP.add)
        wb0, bcomb0 = _emit_gate(nc, pools, 0, pooled0, consts, h_ext0)
        wcombr0 = _emit_mac(nc, pools, 0, wb0, wpsA_sb, wpsB_sb)

        part1 = pools["gate"].tile([128, 2], F32, tag="part", name="part1")
        s1_gap_plan = {
            4: (GAP_TOP[0], False, "dve"),
            7: (GAP_TOP[1], False, "act"),
            10: (GAP_BOT[0], True, "dve"),
            13: (GAP_BOT[1], True, "act"),
        }

        def s1_gap_hook(p):
            if p in s1_gap_plan:
                win, is_bot, eng = s1_gap_plan[p]
                _emit_gap_op(nc, pools, XX1, part1, win, is_bot=is_bot, eng=eng)

        # out batching: batch b (4 pairs, 24 rows; b=5 is 8 rows) goes to
        # partition half b%2 of the [128, OBW] buffer for DMA j=b//2
        obstate = {0: [None, 0], 1: [None, 0]}  # per sample: [tile, rows_in_batch]

        def emit_sample_pairs(s, XX, wcombr, bcomb, rng, hook=None):
            for p in rng:
                batch = min(p // 4, 5)
                half = batch % 2
                j = batch // 2
                ob, orow = obstate[s]
                if ob is None:
                    if half == 0:
                        ob = pools["stage"].tile(
                            [128, OBW], BF16, tag="ob", name=f"ob{s}_{j}", bufs=3
                        )
                    else:
                        ob = obstate[s][0] if obstate[s][0] is not None else None
                    obstate[s] = [ob, 0]
                    orow = 0
                _emit_pair(nc, pools, s, p, XX, wcombr, bcomb, ob, half, orow * WP)
                orow += sum(TILES[t][1] for t in PAIRS[p])
                obstate[s][1] = orow
                brows = 24 if batch < 5 else 8
                if orow == brows:
                    if half == 1 or batch == 5:
                        nc.gpsimd.dma_start(
                            out_ap[s, :, OBW * j : OBW * (j + 1)], ob[:]
                        )
                        obstate[s] = [None, 0]
                    else:
                        obstate[s] = [ob, 0]
                if hook is not None:
                    hook(p)

        emit_sample_pairs(0, XX0, wcombr0, bcomb0, range(GATE_SPLIT), s1_gap_hook)
        pooled1 = pools["gate"].tile([128, 1], F32, tag="pooled", name="pooled1")
        nc.vector.tensor_reduce(pooled1, part1[:], axis=AX.X, op=OP.add)
        wb1, bcomb1 = _emit_gate(nc, pools, 1, pooled1, consts, h_ext1)
        wcombr1 = _emit_mac(nc, pools, 1, wb1, wpsA_sb, wpsB_sb)
        emit_sample_pairs(0, XX0, wcombr0, bcomb0, range(GATE_SPLIT, len(PAIRS)))
        emit_sample_pairs(1, XX1, wcombr1, bcomb1, range(len(PAIRS)))

    nc.compile()
    _cache["nc"] = nc
    return nc


def host_prep(x, wg1, bg1, wg2, bg2, w_exp, b_exp):
    """Host-side layout prep + per-core sharding. Returns in_maps list."""
    x = np.asarray(x, dtype=np.float32)
    wg1 = np.asarray(wg1, dtype=np.float32)
    bg1 = np.asarray(bg1, dtype=np.float32)
    wg2 = np.asarray(wg2, dtype=np.float32)
    bg2 = np.asarray(bg2, dtype=np.float32)
    w_exp = np.asarray(w_exp, dtype=np.float32)
    b_exp = np.asarray(b_exp, dtype=np.float32)

    # x shipped as [B, 128, FLAT] bf16: rows 0:64 = zero-padded flat
    # image, rows 64:128 = the same shifted +2 elements (the conv's
    # bottom-half K copy) — both SBUF halves land in one full-rate DMA
    xpad = np.zeros((B, C, HP, WP), np.float32)
    xpad[:, :, 1 : H + 1, 1 : W + 1] = x
    flat = xpad.reshape(B, C, FLAT)
    xs = np.zeros((B, 128, FLAT), NPBF16)
    xs[:, 0:64] = flat.astype(NPBF16)
    xs[:, 64:128, 0 : FLAT - 2] = flat[:, :, 2:].astype(NPBF16)

    # wps [128, E, 3(dy), 128]: K top/bottom = taps dx 0/2 on M 0:64 (A),
    # center dx=1 on M 64:128 top (B, bottom zero). Residual identity is
    # folded into every expert's center tap (sum of probs is ~1).
    wt = np.transpose(w_exp, (2, 0, 3, 4, 1))  # [I, E, dy, dx, O]
    wps = np.zeros((128, E, 3, 128), np.float32)
    wps[0:64, :, :, 0:64] = wt[:, :, :, 0, :]
    wps[64:128, :, :, 0:64] = wt[:, :, :, 2, :]
    wps[0:64, :, :, 64:128] = wt[:, :, :, 1, :]
    ii = np.arange(64)
    wps[ii, :, 1, 64 + ii] += 1.0

    gconst = np.zeros((128, 90), np.float32)
    gconst[:, 0:16] = np.concatenate([wg1, wg1], axis=0) / (H * W)
    gconst[0:16, 16] = bg1
    gconst[0:16, 17:25] = wg2
    gconst[16, 17:25] = bg2
    gconst[0:8, 25:89] = b_exp

    shared = {
        "wpsA": np.ascontiguousarray(wps[:, 0:4]).astype(NPBF16),
        "wpsB": np.ascontiguousarray(wps[:, 4:8]).astype(NPBF16),
        "gconst": gconst,
    }
    return [
        {"xs": np.ascontiguousarray(xs[SPB * k : SPB * (k + 1)]), **shared}
        for k in range(NCORES)
    ]


def _decode_out(o):
    """[128, 3*OBW] bf16 -> [C, H, W] f32 (strip pads, reassemble batches)."""
    res = np.empty((C, H, W), np.float32)
    for b in range(6):
        j, half = b // 2, b % 2
        rows = 24 if b < 5 else 8
        blk = o[64 * half : 64 * half + 64, OBW * j : OBW * j + rows * WP]
        blk = np.asarray(blk, dtype=np.float32).reshape(C, rows, WP)
        res[:, 24 * b : 24 * b + rows, :] = blk[:, :, 0:W]
    return res


def kernel(x, wg1, bg1, wg2, bg2, w_exp, b_exp):
    nc = build_program()
    in_maps = host_prep(x, wg1, bg1, wg2, bg2, w_exp, b_exp)
    res = run_bass_kernel_spmd(nc, in_maps, list(range(NCORES)))
    out = np.empty((B, C, H, W), np.float32)
    for k in range(NCORES):
        o = np.asarray(res.results[k]["out"])
        for s in range(SPB):
            out[SPB * k + s] = _decode_out(o[s])
    return out


# revision 20
# speedup vs baseline: 1.0569x; 1.0006x over previous
"""MoE block (top-2 routed 3x3 conv experts) Trainium2 Bass kernel — v4.

Data-parallel over batch, 2 samples per core on 8 cores. The conv is
linear in the kernel, so the top-2 expert kernels are combined with the
routing probabilities first (w_comb = sum_e p_e W_e + I, the +I folding
the residual into the center tap), then one 3x3 SAME conv per sample.

Conv-as-matmul, 3 matmuls per 3-row tile: x lives zero-padded
[130x130] per channel in SBUF (bf16), partitions 0-63 = channels,
partitions 64-127 = same channels shifted +2 pixels. lhsT is
[128K, 128M]: M 0:64 ("A") = taps dx=0 (top K) / dx=2 (bottom K);
M 64:128 ("B") = center dx=1 taps (top K, bottom zero). The 3 dy reads
(390 cols each, dyi-major for LDWEIGHTS reuse) accumulate into one
PSUM bank per tile. Post-op: ACT stages psB (+1 col) to SBUF bf16 (DVE
may read only ONE PSUM operand, NCC_IBVF027), then one DVE
scalar_tensor_tensor per 2-tile pair: out = psA + b_comb + sbB.

DMA facts that shape the layout: queues are descriptor-rate-bound
(~4.3ns/desc) AND a transfer spanning only 64 partitions runs at HALF
rate. So the host ships x as [128, 16900] per sample — both SBUF
halves (padded flat + 2-px-shifted copy) prebuilt — making every x
chunk one full-rate 128-partition contiguous DMA (64B-descriptor-free).
The output is likewise written as [128, 9360]: 24-row batches packed
into alternating partition halves (host strips pads / reassembles).

Lanes: SP = s0 x chunks 0-1, wps weights, h_ext consts, out DMAs;
gpsimd = gconst, s0 x chunks 2-3, all s1 x chunks; ACT = s0-top GAP,
exp, B-half stagings; DVE = s0-bot GAP, gates, MAC chains, s1 GAP
(spread between pair combines), pair combines.
"""
import numpy as np
from contextlib import ExitStack

import ml_dtypes

import concourse.bass as bass
import concourse.tile as tile
from concourse import bacc, mybir
from concourse.bass_utils import run_bass_kernel_spmd

F32 = mybir.dt.float32
BF16 = mybir.dt.bfloat16
AX = mybir.AxisListType
OP = mybir.AluOpType
ACTF = mybir.ActivationFunctionType

B, C, H, W, E, GH = 16, 64, 128, 128, 8, 16
NCORES = 8
SPB = B // NCORES          # samples per core
HP, WP = H + 2, W + 2      # 130
FLAT = HP * WP             # 16900
QC = FLAT // 4             # x-load chunk size (4225 flat elements)
OBW = 24 * WP              # out batch region width (3120)
TILES = [(3 * t, 3) for t in range(42)] + [(126, 2)]
PAIRS = [(2 * p, 2 * p + 1) for p in range(21)] + [(42,)]
GATE_SPLIT = 14            # emit s1 gate work after this many s0 pairs

NPBF16 = ml_dtypes.bfloat16

_cache = {}

# GAP windows over the flat layout (pad zeros included): top covers x
# rows 0..63 (+ the first elem of x row 64 that the shifted bottom copy
# misses), bottom covers x rows 64..127.
GAP_TOP = [(0, QC, 0), (QC, 2 * QC + 2, 1)]
GAP_BOT = [(2 * QC, 3 * QC, 0), (3 * QC, FLAT, 1)]


def _emit_gap_op(nc, pools, XX, part, win, is_bot, eng):
    a, b, slot = win
    lo, hi = (64, 128) if is_bot else (0, 64)
    src = XX[lo:hi, a:b]
    dst = pools["scrS" if eng == "act" else "scrD"][lo:hi]
    acc = part[lo:hi, slot : slot + 1]
    if eng == "act":
        return nc.scalar.activation(
            dst[:, 0 : b - a], src, ACTF.Copy, accum_out=acc
        )
    return nc.vector.tensor_scalar(
        dst[:, 0 : b - a], src, 0.0, 0.0, OP.add, OP.add, accum_out=acc
    )


def _emit_gate(nc, pools, s, pooled, consts, h_ext):
    """Gate MLP + softmax + top-2 for one sample (all f32).

    exp-without-max-sub (logits are small); folds the top-2 mask and
    renormalization: w8 = (u>=m2)*u / (sum((u>=m2)*u) + sum(u)*1e-8).
    Returns (wb_sb [128,E] f32 per-partition probs, b_comb [C,1]).
    """
    f = pools
    g = f["gate"]
    wg1x2_sb, bg1_sb, wg2_sb, bexp_sb, ones = consts
    n = lambda base: f"{base}{s}"

    h_ps = f["cpsum"].tile([GH, 1], F32, tag="cps", name=n("h_ps"))
    nc.tensor.matmul(h_ps[:], lhsT=wg1x2_sb, rhs=pooled[:], start=True, stop=True)
    nc.vector.tensor_scalar(h_ext[0:GH, :], h_ps[:], bg1_sb, 0.0, OP.add, OP.max)

    lg_ps = f["cpsum"].tile([1, E], F32, tag="cps", name=n("lg_ps"))
    nc.tensor.matmul(lg_ps[:], lhsT=h_ext[:], rhs=wg2_sb, start=True, stop=True)

    u = g.tile([1, E], F32, tag="u", name=n("u"))
    nc.scalar.activation(u[:], lg_ps[:], ACTF.Exp)
    usum = g.tile([1, 1], F32, tag="usum", name=n("usum"))
    nc.vector.tensor_reduce(usum[:], u[:], axis=AX.X, op=OP.add)
    m1p = g.tile([1, 1], F32, tag="m1p", name=n("m1p"))
    nc.vector.tensor_reduce(m1p[:], u[:], axis=AX.X, op=OP.max)
    pm = g.tile([1, E], F32, tag="pm", name=n("pm"))
    nc.vector.scalar_tensor_tensor(pm[:], u[:], m1p[:], u[:], op0=OP.is_lt, op1=OP.mult)
    m2 = g.tile([1, 1], F32, tag="m2", name=n("m2"))
    nc.vector.tensor_reduce(m2[:], pm[:], axis=AX.X, op=OP.max)
    spv = g.tile([1, E], F32, tag="spv", name=n("spv"))
    nc.vector.scalar_tensor_tensor(spv[:], u[:], m2[:], u[:], op0=OP.is_ge, op1=OP.mult)
    dsum = g.tile([1, 1], F32, tag="dsum", name=n("dsum"))
    nc.vector.tensor_reduce(dsum[:], spv[:], axis=AX.X, op=OP.add)
    dd = g.tile([1, 1], F32, tag="dd", name=n("dd"))
    nc.vector.scalar_tensor_tensor(dd[:], usum[:], 1e-8, dsum[:], op0=OP.mult, op1=OP.add)
    rr = g.tile([1, 1], F32, tag="rr", name=n("rr"))
    nc.vector.reciprocal(rr[:], dd[:])
    w8 = g.tile([1, E], F32, tag="w8", name=n("w8"))
    nc.vector.tensor_scalar_mul(w8[:], spv[:], rr[:])

    # broadcast w8 down all 128 partitions, then stage to SBUF for MACs
    wb_ps = f["cpsum"].tile([128, E], F32, tag="cps", name=n("wb_ps"))
    nc.tensor.matmul(wb_ps[:], lhsT=ones[:], rhs=w8[:], start=True, stop=True)
    wb_sb = g.tile([128, E], F32, tag="wb_sb", name=n("wb_sb"))
    nc.vector.tensor_copy(wb_sb[:], wb_ps[:])

    # combined bias: b_comb = b_exp^T @ w8^T
    w8c_ps = f["cpsum"].tile([E, 1], F32, tag="cps", name=n("w8c_ps"))
    nc.tensor.matmul(w8c_ps[:], lhsT=w8[:], rhs=ones[:, 0:1], start=True, stop=True)
    w8col = g.tile([E, 1], F32, tag="w8col", name=n("w8col"))
    nc.vector.tensor_copy(w8col[:], w8c_ps[:])
    bc_ps = f["cpsum"].tile([C, 1], F32, tag="cps", name=n("bc_ps"))
    nc.tensor.matmul(bc_ps[:], lhsT=bexp_sb, rhs=w8col[:], start=True, stop=True)
    b_comb = g.tile([C, 1], F32, tag="b_comb", name=n("b_comb"))
    nc.vector.tensor_copy(b_comb[:], bc_ps[:])
    return wb_sb, b_comb


def _emit_mac(nc, pools, s, wb_sb, wpsA_sb, wpsB_sb):
    """wcomb = sum_e p_e wps_e: single DVE MAC chain accumulating in bf16.
    Residual identity is pre-folded into every expert's center-tap B-half
    on the host."""
    f = pools
    wcombr = f["wcomb"].tile([128, 3, 128], BF16, tag="wcombr", name=f"wcombr{s}")
    nc.vector.tensor_scalar_mul(wcombr[:], wpsA_sb[:, 0], wb_sb[:, 0:1])
    for e in range(1, E):
        src = wpsA_sb[:, e] if e < 4 else wpsB_sb[:, e - 4]
        nc.vector.scalar_tensor_tensor(
            wcombr[:], src, wb_sb[:, e : e + 1], wcombr[:],
            op0=OP.mult, op1=OP.add,
        )
    return wcombr


def _emit_pair(nc, pools, s, p, XX, wcombr, b_comb, ob, half, ocol):
    """Conv for tile pair p: 6 (or 3) matmuls into a 2-bank PSUM tile,
    dyi-major so consecutive matmuls share lhsT. ACT stages the B half
    (+1 col) to SBUF bf16, DVE combines into the out batch buffer
    (partition half selected by batch parity)."""
    f = pools
    XX3 = XX[:, 0:FLAT].rearrange("p (r c) -> p r c", c=WP)
    tl = PAIRS[p]
    ps = f["cpsum"].tile([128, 2, 512], F32, tag="cps", name=f"cps{s}_{p}")
    for dyi in range(3):
        for t01, t in enumerate(tl):
            r0, nr = TILES[t]
            nc.tensor.matmul(
                ps[:, t01, 0 : nr * WP],
                lhsT=wcombr[:, dyi, :],
                rhs=XX3[:, r0 + dyi : r0 + dyi + nr, :],
                start=(dyi == 0),
                stop=(dyi == 2),
            )
    lo = 64 * half
    if len(tl) == 2:
        sbB = f["stage"].tile([128, 2, 390], BF16, tag="sbB", name=f"sbB{s}_{p}")
        nc.scalar.activation(sbB[0:64], ps[64:128, :, 1:391], ACTF.Copy)
        return nc.vector.scalar_tensor_tensor(
            ob[lo : lo + 64, ocol : ocol + 780].rearrange("p (t c) -> p t c", c=390),
            ps[0:64, :, 0:390],
            b_comb[:],
            sbB[0:64],
            op0=OP.add,
            op1=OP.add,
        )
    else:
        ncols = TILES[tl[0]][1] * WP
        sbB = f["stage"].tile([128, 2, 390], BF16, tag="sbB", name=f"sbB{s}_{p}")
        nc.scalar.activation(
            sbB[0:64, 0, 0:ncols], ps[64:128, 0, 1 : 1 + ncols], ACTF.Copy
        )
        return nc.vector.scalar_tensor_tensor(
            ob[lo : lo + 64, ocol : ocol + ncols],
            ps[0:64, 0, 0:ncols],
            b_comb[:],
            sbB[0:64, 0, 0:ncols],
            op0=OP.add,
            op1=OP.add,
        )


def build_program():
    if "nc" in _cache:
        return _cache["nc"]
    nc = bacc.Bacc("TRN2", target_bir_lowering=False, debug=False, enable_asserts=False)
    xs_ap = nc.dram_tensor("xs", [SPB, 128, FLAT], BF16, kind="ExternalInput").ap()
    wpsA_d = nc.dram_tensor("wpsA", [128, E // 2, 3, 128], BF16, kind="ExternalInput").ap()
    wpsB_d = nc.dram_tensor("wpsB", [128, E // 2, 3, 128], BF16, kind="ExternalInput").ap()
    gconst_d = nc.dram_tensor("gconst", [128, 90], F32, kind="ExternalInput").ap()
    out_ap = nc.dram_tensor("out", [SPB, 128, 3 * OBW], BF16, kind="ExternalOutput").ap()

    with tile.TileContext(nc) as tc, ExitStack() as ctx:
        pools = {
            "const": ctx.enter_context(tc.tile_pool(name="const", bufs=1)),
            "xx": ctx.enter_context(tc.tile_pool(name="xx", bufs=SPB)),
            "gate": ctx.enter_context(tc.tile_pool(name="gate", bufs=2)),
            "wcomb": ctx.enter_context(tc.tile_pool(name="wcomb", bufs=2)),
            "stage": ctx.enter_context(tc.tile_pool(name="stage", bufs=6)),
            "cpsum": ctx.enter_context(tc.tile_pool(name="cpsum", bufs=4, space="PSUM")),
        }
        cp = pools["const"]
        XX0 = pools["xx"].tile([128, FLAT], BF16, tag="XX", name="XX0")
        XX1 = pools["xx"].tile([128, FLAT], BF16, tag="XX", name="XX1")
        gconst_sb = cp.tile([128, 90], F32)
        nc.gpsimd.dma_start(gconst_sb[:], gconst_d[:])
        ones = cp.tile([1, 128], F32)
        nc.gpsimd.memset(ones[:], 1.0)
        warm = cp.tile([1, 1], F32)
        nc.scalar.activation(warm[:], ones[:, 0:1], ACTF.Exp)
        wpsA_sb = cp.tile([128, E // 2, 3, 128], BF16)
        wpsB_sb = cp.tile([128, E // 2, 3, 128], BF16)
        pools["scrD"] = cp.tile([128, QC + 2], BF16, name="scrD")
        pools["scrS"] = cp.tile([128, QC + 2], BF16, name="scrS")

        wg1x2_sb = gconst_sb[:, 0:16]
        bg1_sb = gconst_sb[0:16, 16:17]
        wg2_sb = gconst_sb[0:17, 17:25]
        bexp_sb = gconst_sb[0:8, 25:89]
        consts = (wg1x2_sb, bg1_sb, wg2_sb, bexp_sb, ones)

        # ---- x loads: full-rate 128-partition contiguous chunks ----
        # SP and gpsimd are pure DMA lanes (ACT stays compute-only so GAP
        # ops and stagings never queue behind transfers): SP = s0 chunks
        # 0,1 + weights + all of s1; gps = s0 chunks 2,3 + out DMAs
        for q in (0, 1):
            nc.sync.dma_start(
                XX0[:, QC * q : QC * (q + 1)], xs_ap[0, :, QC * q : QC * (q + 1)]
            )
        for q in (2, 3):
            nc.gpsimd.dma_start(
                XX0[:, QC * q : QC * (q + 1)], xs_ap[0, :, QC * q : QC * (q + 1)]
            )
        nc.sync.dma_start(wpsA_sb[:], wpsA_d[:])
        nc.sync.dma_start(wpsB_sb[:], wpsB_d[:])
        # gate h_ext tiles (trailing 1.0 row loaded once from `ones`)
        h_ext0 = pools["gate"].tile([GH + 1, 1], F32, tag="h_ext", name="h_ext0")
        h_ext1 = pools["gate"].tile([GH + 1, 1], F32, tag="h_ext", name="h_ext1")
        nc.sync.dma_start(h_ext0[GH : GH + 1, 0:1], ones[0:1, 0:1])
        nc.sync.dma_start(h_ext1[GH : GH + 1, 0:1], ones[0:1, 0:1])
        for q in range(4):
            nc.sync.dma_start(
                XX1[:, QC * q : QC * (q + 1)], xs_ap[1, :, QC * q : QC * (q + 1)]
            )

        # ---- s0 GAP: tops on ACT, bottoms on DVE ----
        part0 = pools["gate"].tile([128, 2], F32, tag="part", name="part0")
        for win in GAP_TOP:
            _emit_gap_op(nc, pools, XX0, part0, win, is_bot=False, eng="act")
        for win in GAP_BOT:
            _emit_gap_op(nc, pools, XX0, part0, win, is_bot=True, eng="dve")
        pooled0 = pools["gate"].tile([128, 1], F32, tag="pooled", name="pooled0")
        nc.vector.tensor_reduce(pooled0, part0[:], axis=AX.X, op=OP.add)
        wb0, bcomb0 = _emit_gate(nc, pools, 0, pooled0, consts, h_ext0)
        wcombr0 = _emit_mac(nc, pools, 0, wb0, wpsA_sb, wpsB_sb)

        part1 = pools["gate"].tile([128, 2], F32, tag="part", name="part1")
        s1_gap_plan = {
            4: (GAP_TOP[0], False, "dve"),
            7: (GAP_TOP[1], False, "act"),
            10: (GAP_BOT[0], True, "dve"),
            13: (GAP_BOT[1], True, "act"),
        }

        def s1_gap_hook(p, comb):
            if p in s1_gap_plan:
                win, is_bot, eng = s1_gap_plan[p]
                gi = _emit_gap_op(nc, pools, XX1, part1, win, is_bot=is_bot, eng=eng)
                tile.add_dep_helper(
                    gi.ins, comb.ins, sync=False,
                    reason="s1 GAP slotted after this pair's combine",
                )

        # out batching: batch b (4 pairs, 24 rows; b=5 is 8 rows) goes to
        # partition half b%2 of the [128, OBW] buffer for DMA j=b//2
        obstate = {0: [None, 0], 1: [None, 0]}  # per sample: [tile, rows_in_batch]

        def emit_sample_pairs(s, XX, wcombr, bcomb, rng, hook=None):
            for p in rng:
                batch = min(p // 4, 5)
                half = batch % 2
                j = batch // 2
                ob, orow = obstate[s]
                if ob is None:
                    if half == 0:
                        ob = pools["stage"].tile(
                            [128, OBW], BF16, tag="ob", name=f"ob{s}_{j}", bufs=3
                        )
                    else:
                        ob = obstate[s][0] if obstate[s][0] is not None else None
                    obstate[s] = [ob, 0]
                    orow = 0
                comb = _emit_pair(nc, pools, s, p, XX, wcombr, bcomb, ob, half, orow * WP)
                orow += sum(TILES[t][1] for t in PAIRS[p])
                obstate[s][1] = orow
                brows = 24 if batch < 5 else 8
                if orow == brows:
                    if half == 1 or batch == 5:
                        nc.gpsimd.dma_start(
                            out_ap[s, :, OBW * j : OBW * (j + 1)], ob[:]
                        )
                        obstate[s] = [None, 0]
                    else:
                        obstate[s] = [ob, 0]
                if hook is not None:
                    hook(p, comb)

        emit_sample_pairs(0, XX0, wcombr0, bcomb0, range(GATE_SPLIT), s1_gap_hook)
        pooled1 = pools["gate"].tile([128, 1], F32, tag="pooled", name="pooled1")
        nc.vector.tensor_reduce(pooled1, part1[:], axis=AX.X, op=OP.add)
        wb1, bcomb1 = _emit_gate(nc, pools, 1, pooled1, consts, h_ext1)
        wcombr1 = _emit_mac(nc, pools, 1, wb1, wpsA_sb, wpsB_sb)
        emit_sample_pairs(0, XX0, wcombr0, bcomb0, range(GATE_SPLIT, len(PAIRS)))
        emit_sample_pairs(1, XX1, wcombr1, bcomb1, range(len(PAIRS)))

    nc.compile()
    _cache["nc"] = nc
    return nc


def host_prep(x, wg1, bg1, wg2, bg2, w_exp, b_exp):
    """Host-side layout prep + per-core sharding. Returns in_maps list."""
    x = np.asarray(x, dtype=np.float32)
    wg1 = np.asarray(wg1, dtype=np.float32)
    bg1 = np.asarray(bg1, dtype=np.float32)
    wg2 = np.asarray(wg2, dtype=np.float32)
    bg2 = np.asarray(bg2, dtype=np.float32)
    w_exp = np.asarray(w_exp, dtype=np.float32)
    b_exp = np.asarray(b_exp, dtype=np.float32)

    # x shipped as [B, 128, FLAT] bf16: rows 0:64 = zero-padded flat
    # image, rows 64:128 = the same shifted +2 elements (the conv's
    # bottom-half K copy) — both SBUF halves land in one full-rate DMA
    xpad = np.zeros((B, C, HP, WP), np.float32)
    xpad[:, :, 1 : H + 1, 1 : W + 1] = x
    flat = xpad.reshape(B, C, FLAT)
    xs = np.zeros((B, 128, FLAT), NPBF16)
    xs[:, 0:64] = flat.astype(NPBF16)
    xs[:, 64:128, 0 : FLAT - 2] = flat[:, :, 2:].astype(NPBF16)

    # wps [128, E, 3(dy), 128]: K top/bottom = taps dx 0/2 on M 0:64 (A),
    # center dx=1 on M 64:128 top (B, bottom zero). Residual identity is
    # folded into every expert's center tap (sum of probs is ~1).
    wt = np.transpose(w_exp, (2, 0, 3, 4, 1))  # [I, E, dy, dx, O]
    wps = np.zeros((128, E, 3, 128), np.float32)
    wps[0:64, :, :, 0:64] = wt[:, :, :, 0, :]
    wps[64:128, :, :, 0:64] = wt[:, :, :, 2, :]
    wps[0:64, :, :, 64:128] = wt[:, :, :, 1, :]
    ii = np.arange(64)
    wps[ii, :, 1, 64 + ii] += 1.0

    gconst = np.zeros((128, 90), np.float32)
    gconst[:, 0:16] = np.concatenate([wg1, wg1], axis=0) / (H * W)
    gconst[0:16, 16] = bg1
    gconst[0:16, 17:25] = wg2
    gconst[16, 17:25] = bg2
    gconst[0:8, 25:89] = b_exp

    shared = {
        "wpsA": np.ascontiguousarray(wps[:, 0:4]).astype(NPBF16),
        "wpsB": np.ascontiguousarray(wps[:, 4:8]).astype(NPBF16),
        "gconst": gconst,
    }
    return [
        {"xs": np.ascontiguousarray(xs[SPB * k : SPB * (k + 1)]), **shared}
        for k in range(NCORES)
    ]


def _decode_out(o):
    """[128, 3*OBW] bf16 -> [C, H, W] f32 (strip pads, reassemble batches)."""
    res = np.empty((C, H, W), np.float32)
    for b in range(6):
        j, half = b // 2, b % 2
        rows = 24 if b < 5 else 8
        blk = o[64 * half : 64 * half + 64, OBW * j : OBW * j + rows * WP]
        blk = np.asarray(blk, dtype=np.float32).reshape(C, rows, WP)
        res[:, 24 * b : 24 * b + rows, :] = blk[:, :, 0:W]
    return res


def kernel(x, wg1, bg1, wg2, bg2, w_exp, b_exp):
    nc = build_program()
    in_maps = host_prep(x, wg1, bg1, wg2, bg2, w_exp, b_exp)
    res = run_bass_kernel_spmd(nc, in_maps, list(range(NCORES)))
    out = np.empty((B, C, H, W), np.float32)
    for k in range(NCORES):
        o = np.asarray(res.results[k]["out"])
        for s in range(SPB):
            out[SPB * k + s] = _decode_out(o[s])
    return out
